# revision 17
# baseline (speedup 1.0000x reference)
"""HalfKP input layer (dual GEMV + bias + relu) on 8 Trainium2 NeuronCores.

out[512] = concat(relu(W_my @ x[:41024] + b_my), relu(W_opp @ x[41024:] + b_opp))

Memory-roofline kernel: the 84 MB f32 weight stream is the whole cost.
Weights ship as int8 with one global scale per side (5.5e-3 end-to-end rel
err) and each core's 2.7 MB shard is expanded int8 -> fp16 in flight by
the SWDGE cast-DMA: HBM reads halve (killing cross-core HBM-stack
contention) and the stream is bound only by the core-private SBUF AXI
write side (~435 GB/s, ~12.4 us).

Sharding: k-parallel.  Every core holds all 512 output rows and 1/8 of
the contraction (5128 k per side, zero-padded to 41 blocks of 128).
Compute rides the PE in GEMV mode with fat moving tiles: for k-block g
and side s, matmul(stationary = x_block [128,1], moving = W_block^T
[128,256]) accumulates into a per-side PSUM bank.  Dep-free warm-up
matmuls flip the PE HAM clock-gate to 2.4 GHz during DMA spin-up so real
matmuls run at ~110 ns, fully hidden under the stream.  Cores return raw
f32 integer-unit partials; the host applies the int8 scales, sums the 8
partials, adds bias, applies relu.
"""

import numpy as np

K = 41024              # features per side
KSL = K // 8           # 5128 contraction elems per core per side
NBS = 41               # 128-elem k-blocks per side per core (5248, padded)
BK = NBS * 128
N_CORES = 8
# W chunks in block-pairs (one pair = my+opp blocks = 1 KB/partition fp16
# in SBUF).  The first chunks ride the sync HWDGE ring as integer-valued
# fp16 (data flowing ~2 us before the SWDGE Q7 finishes descriptor-gen);
# the rest stream as int8 through the SWDGE cast-DMA.  Small tail chunk
# keeps the last-semaphore exposure off the critical path.
SYNC_PAIRS = []              # fp16-direct chunks (HWDGE) — mixing rings
                             # delays SWDGE chunk semaphores; keep empty
SWDGE_PAIRS = [4, 8, 8, 8, 8, 4, 1]  # int8->fp16 cast chunks (SWDGE)
                                     # (>8-pair SWDGE chunks wedge the device)
CHUNK_PAIRS = SYNC_PAIRS + SWDGE_PAIRS
assert sum(CHUNK_PAIRS) == NBS
N_WARMUP = 9  # dummy N=512 matmuls that flip the PE HAM clock-gate to
              # 2.4 GHz during DMA spin-up, so real matmuls run warm

_compiled = None


def _build_nc():
    import concourse.bacc as bacc
    import concourse.mybir as mybir
    import concourse.tile as tile

    F32 = mybir.dt.float32
    F16 = mybir.dt.float16
    I8 = mybir.dt.int8

    nc = bacc.Bacc("TRN2", target_bir_lowering=False, debug=False)

    wt_d = [
        nc.dram_tensor(
            f"wt{c}",
            [128, pairs * 512],
            F16 if c < len(SYNC_PAIRS) else I8,
            kind="ExternalInput",
        )
        for c, pairs in enumerate(CHUNK_PAIRS)
    ]
    xq_d = nc.dram_tensor("xq", [128, 2 * NBS], F16, kind="ExternalInput")
    outA_d = nc.dram_tensor("outA", [1, 256], F32, kind="ExternalOutput")
    outB_d = nc.dram_tensor("outB", [1, 256], F32, kind="ExternalOutput")

    with tile.TileContext(nc) as tc:
        with (
            tc.tile_pool(name="const", bufs=1) as constp,
            tc.tile_pool(name="w", bufs=len(CHUNK_PAIRS) + 1) as wp,
            tc.tile_pool(name="ps", bufs=1, space="PSUM") as psp,
        ):
            # x rides the scalar (ACT) HWDGE ring; the W stream runs on the
            # SWDGE (gpsimd) ring which also does the int8->fp16 expansion
            xq = constp.tile([128, 2 * NBS], F16, tag="xq")
            nc.scalar.dma_start(xq[:], xq_d[:])

            # full-bank tiles so the two accumulation groups can't share a
            # bank (a group's start=True clears its whole bank)
            psA = psp.tile([1, 512], F32, tag="psA")
            psB = psp.tile([1, 512], F32, tag="psB")

            # PE warm-up: dep-free matmuls on zeroed scratch keep the PE
            # busy from t=0 so HAM un-throttles before real work arrives
            warm_w = constp.tile([128, 512], F16, tag="warm_w")
            warm_x = constp.tile([128, 1], F16, tag="warm_x")
            warm_ps = psp.tile([1, 512], F32, tag="warm_ps")
            nc.vector.memset(warm_w[:], 0)
            nc.vector.memset(warm_x[:], 0)
            for _ in range(N_WARMUP):
                nc.tensor.matmul(
                    warm_ps[:], lhsT=warm_x[:], rhs=warm_w[:],
                    start=True, stop=True,
                )

            g = 0
            for c, pairs in enumerate(CHUNK_PAIRS):
                w_sb = wp.tile([128, pairs * 512], F16, tag="w")
                if c < len(SYNC_PAIRS):
                    nc.sync.dma_start(w_sb[:], wt_d[c][:])
                else:
                    nc.gpsimd.dma_start(w_sb[:], wt_d[c][:])  # int8 -> fp16
                for j in range(pairs):
                    for s, ps in ((0, psA), (1, psB)):
                        nc.tensor.matmul(
                            ps[:, 0:256],
                            lhsT=xq[:, 2 * (g + j) + s : 2 * (g + j) + s + 1],
                            rhs=w_sb[:, j * 512 + s * 256 : j * 512 + (s + 1) * 256],
                            start=(g + j == 0),
                            stop=(g + j == NBS - 1),
                        )
                g += pairs

            # different PSUM banks -> ScalarE and VectorE copy in parallel,
            # then each half goes out on its own HWDGE ring so the two
            # completion receipts overlap
            outA_sb = constp.tile([1, 256], F32, tag="outA")
            outB_sb = constp.tile([1, 256], F32, tag="outB")
            nc.scalar.copy(outA_sb[:], psA[:, 0:256])
            nc.vector.tensor_scalar_add(outB_sb[:], psB[:, 0:256], 0.0)
            nc.scalar.dma_start(outA_d[:], outA_sb[:])
            nc.sync.dma_start(outB_d[:], outB_sb[:])

    nc.compile()
    return nc


def _get_nc():
    global _compiled
    if _compiled is None:
        _compiled = _build_nc()
    return _compiled


def _quant_scales(W_my, W_opp):
    return (
        np.abs(np.asarray(W_my, np.float32)).max() / 127.0,
        np.abs(np.asarray(W_opp, np.float32)).max() / 127.0,
    )


def make_in_maps(input, W_my, b_my, W_opp, b_opp):
    """Host-side sharding: per-core input dicts."""
    x = np.asarray(input, np.float32)
    s_my, s_opp = _quant_scales(W_my, W_opp)
    Wq = [
        np.round(np.asarray(W_my, np.float32) / s_my).astype(np.int8),
        np.round(np.asarray(W_opp, np.float32) / s_opp).astype(np.int8),
    ]
    xs = [x[:K], x[K:]]

    in_maps = []
    for core in range(N_CORES):
        ksl = slice(core * KSL, (core + 1) * KSL)
        # wt[p, (2g+s)*256 + j] = Wq_s[j, core*KSL + g*128 + p]
        wt = np.zeros((128, NBS, 2, 256), np.int8)
        xq = np.zeros((128, NBS, 2), np.float16)
        for s in (0, 1):
            Wp = np.zeros((BK, 256), np.int8)
            Wp[:KSL] = Wq[s][:, ksl].T  # [KSL, 256]
            wt[:, :, s, :] = Wp.reshape(NBS, 128, 256).transpose(1, 0, 2)
            xp = np.zeros(BK, np.float16)
            xp[:KSL] = xs[s][ksl]
            xq[:, :, s] = xp.reshape(NBS, 128).T
        wt = wt.reshape(128, NBS * 512)
        im = {"xq": np.ascontiguousarray(xq.reshape(128, 2 * NBS))}
        g = 0
        for c, pairs in enumerate(CHUNK_PAIRS):
            sl = wt[:, g * 512 : (g + pairs) * 512]
            if c < len(SYNC_PAIRS):
                sl = sl.astype(np.float16)  # integer-valued, exact
            im[f"wt{c}"] = np.ascontiguousarray(sl)
            g += pairs
        in_maps.append(im)
    return in_maps


def gather_output(results, W_my, b_my, W_opp, b_opp):
    """results: per-core {'outA','outB': [1,256]} int-unit partials."""
    s_my, s_opp = _quant_scales(W_my, W_opp)
    acc = np.zeros(512, np.float32)
    for core in range(N_CORES):
        acc[:256] += np.asarray(results[core]["outA"], np.float32)[0]
        acc[256:] += np.asarray(results[core]["outB"], np.float32)[0]
    acc[:256] *= s_my
    acc[256:] *= s_opp
    bcat = np.concatenate(
        [np.asarray(b_my, np.float32), np.asarray(b_opp, np.float32)]
    )
    return np.maximum(acc + bcat, 0.0)


def run_on_hw(in_maps, trace=False, **kwargs):
    from concourse.bass_utils import run_bass_kernel_spmd

    nc = _get_nc()
    return run_bass_kernel_spmd(
        nc, in_maps, core_ids=list(range(N_CORES)), trace=trace, **kwargs
    )


def kernel(input, W_my, b_my, W_opp, b_opp):
    in_maps = make_in_maps(input, W_my, b_my, W_opp, b_opp)
    res = run_on_hw(in_maps)
    return gather_output(res.results, W_my, b_my, W_opp, b_opp)


# revision 18
# speedup vs baseline: 1.0266x; 1.0266x over previous
"""HalfKP input layer (dual GEMV + bias + relu) on 8 Trainium2 NeuronCores.

out[512] = concat(relu(W_my @ x[:41024] + b_my), relu(W_opp @ x[41024:] + b_opp))

Memory-roofline kernel: the 84 MB f32 weight stream is the whole cost.
Weights ship as int8 with one global scale per side (5.5e-3 end-to-end rel
err) and each core's 2.7 MB shard is expanded int8 -> fp16 in flight by
the SWDGE cast-DMA: HBM reads halve (killing cross-core HBM-stack
contention) and the stream is bound only by the core-private SBUF AXI
write side (~435 GB/s, ~12.4 us).

Sharding: k-parallel.  Every core holds all 512 output rows and 1/8 of
the contraction (5128 k per side, zero-padded to 41 blocks of 128).
Compute rides the PE in GEMV mode with fat moving tiles: for k-block g
and side s, matmul(stationary = x_block [128,1], moving = W_block^T
[128,256]) accumulates into a per-side PSUM bank.  Dep-free warm-up
matmuls flip the PE HAM clock-gate to 2.4 GHz during DMA spin-up so real
matmuls run at ~110 ns, fully hidden under the stream.  Cores return raw
f32 integer-unit partials; the host applies the int8 scales, sums the 8
partials, adds bias, applies relu.
"""

import numpy as np

K = 41024              # features per side
KSL = K // 8           # 5128 contraction elems per core per side
NBS = 41               # 128-elem k-blocks per side per core (5248, padded)
BK = NBS * 128
N_CORES = 8
# W chunks in block-pairs (one pair = my+opp blocks = 1 KB/partition fp16
# in SBUF).  The first chunks ride the sync HWDGE ring as integer-valued
# fp16 (data flowing ~2 us before the SWDGE Q7 finishes descriptor-gen);
# the rest stream as int8 through the SWDGE cast-DMA.  Small tail chunk
# keeps the last-semaphore exposure off the critical path.
SYNC_PAIRS = [4]             # fp16-direct chunks (HWDGE, early start)
SWDGE_PAIRS = [8, 8, 8, 8, 4, 1]  # int8->fp16 cast chunks (SWDGE)
                                  # (>8-pair SWDGE chunks wedge the device)
CHUNK_PAIRS = SYNC_PAIRS + SWDGE_PAIRS
assert sum(CHUNK_PAIRS) == NBS
N_WARMUP = 9  # dummy N=512 matmuls that flip the PE HAM clock-gate to
              # 2.4 GHz during DMA spin-up, so real matmuls run warm

_compiled = None


def _build_nc():
    import concourse.bacc as bacc
    import concourse.mybir as mybir
    import concourse.tile as tile

    F32 = mybir.dt.float32
    F16 = mybir.dt.float16
    I8 = mybir.dt.int8

    nc = bacc.Bacc("TRN2", target_bir_lowering=False, debug=False)

    wt_d = [
        nc.dram_tensor(
            f"wt{c}",
            [128, pairs * 512],
            F16 if c < len(SYNC_PAIRS) else I8,
            kind="ExternalInput",
        )
        for c, pairs in enumerate(CHUNK_PAIRS)
    ]
    xq_d = nc.dram_tensor("xq", [128, 2 * NBS], F16, kind="ExternalInput")
    outA_d = nc.dram_tensor("outA", [1, 256], F32, kind="ExternalOutput")
    outB_d = nc.dram_tensor("outB", [1, 256], F32, kind="ExternalOutput")

    with tile.TileContext(nc) as tc:
        with (
            tc.tile_pool(name="const", bufs=1) as constp,
            tc.tile_pool(name="w", bufs=len(CHUNK_PAIRS) + 1) as wp,
            tc.tile_pool(name="ps", bufs=1, space="PSUM") as psp,
        ):
            # x rides the scalar (ACT) HWDGE ring; the W stream runs on the
            # SWDGE (gpsimd) ring which also does the int8->fp16 expansion
            xq = constp.tile([128, 2 * NBS], F16, tag="xq")
            nc.scalar.dma_start(xq[:], xq_d[:])

            # full-bank tiles so the two accumulation groups can't share a
            # bank (a group's start=True clears its whole bank)
            psA = psp.tile([1, 512], F32, tag="psA")
            psB = psp.tile([1, 512], F32, tag="psB")

            # PE warm-up: dep-free matmuls on zeroed scratch keep the PE
            # busy from t=0 so HAM un-throttles before real work arrives
            warm_w = constp.tile([128, 512], F16, tag="warm_w")
            warm_x = constp.tile([128, 1], F16, tag="warm_x")
            warm_ps = psp.tile([1, 512], F32, tag="warm_ps")
            nc.vector.memset(warm_w[:], 0)
            nc.vector.memset(warm_x[:], 0)
            for _ in range(N_WARMUP):
                nc.tensor.matmul(
                    warm_ps[:], lhsT=warm_x[:], rhs=warm_w[:],
                    start=True, stop=True,
                )

            g = 0
            for c, pairs in enumerate(CHUNK_PAIRS):
                w_sb = wp.tile([128, pairs * 512], F16, tag="w")
                if c < len(SYNC_PAIRS):
                    nc.sync.dma_start(w_sb[:], wt_d[c][:])
                else:
                    nc.gpsimd.dma_start(w_sb[:], wt_d[c][:])  # int8 -> fp16
                for j in range(pairs):
                    for s, ps in ((0, psA), (1, psB)):
                        nc.tensor.matmul(
                            ps[:, 0:256],
                            lhsT=xq[:, 2 * (g + j) + s : 2 * (g + j) + s + 1],
                            rhs=w_sb[:, j * 512 + s * 256 : j * 512 + (s + 1) * 256],
                            start=(g + j == 0),
                            stop=(g + j == NBS - 1),
                        )
                g += pairs

            # different PSUM banks -> ScalarE and VectorE copy in parallel,
            # then each half goes out on its own HWDGE ring so the two
            # completion receipts overlap
            outA_sb = constp.tile([1, 256], F32, tag="outA")
            outB_sb = constp.tile([1, 256], F32, tag="outB")
            nc.scalar.copy(outA_sb[:], psA[:, 0:256])
            nc.vector.tensor_scalar_add(outB_sb[:], psB[:, 0:256], 0.0)
            nc.scalar.dma_start(outA_d[:], outA_sb[:])
            nc.sync.dma_start(outB_d[:], outB_sb[:])

    nc.compile()
    return nc


def _get_nc():
    global _compiled
    if _compiled is None:
        _compiled = _build_nc()
    return _compiled


def _quant_scales(W_my, W_opp):
    return (
        np.abs(np.asarray(W_my, np.float32)).max() / 127.0,
        np.abs(np.asarray(W_opp, np.float32)).max() / 127.0,
    )


def make_in_maps(input, W_my, b_my, W_opp, b_opp):
    """Host-side sharding: per-core input dicts."""
    x = np.asarray(input, np.float32)
    s_my, s_opp = _quant_scales(W_my, W_opp)
    Wq = [
        np.round(np.asarray(W_my, np.float32) / s_my).astype(np.int8),
        np.round(np.asarray(W_opp, np.float32) / s_opp).astype(np.int8),
    ]
    xs = [x[:K], x[K:]]

    in_maps = []
    for core in range(N_CORES):
        ksl = slice(core * KSL, (core + 1) * KSL)
        # wt[p, (2g+s)*256 + j] = Wq_s[j, core*KSL + g*128 + p]
        wt = np.zeros((128, NBS, 2, 256), np.int8)
        xq = np.zeros((128, NBS, 2), np.float16)
        for s in (0, 1):
            Wp = np.zeros((BK, 256), np.int8)
            Wp[:KSL] = Wq[s][:, ksl].T  # [KSL, 256]
            wt[:, :, s, :] = Wp.reshape(NBS, 128, 256).transpose(1, 0, 2)
            xp = np.zeros(BK, np.float16)
            xp[:KSL] = xs[s][ksl]
            xq[:, :, s] = xp.reshape(NBS, 128).T
        wt = wt.reshape(128, NBS * 512)
        im = {"xq": np.ascontiguousarray(xq.reshape(128, 2 * NBS))}
        g = 0
        for c, pairs in enumerate(CHUNK_PAIRS):
            sl = wt[:, g * 512 : (g + pairs) * 512]
            if c < len(SYNC_PAIRS):
                sl = sl.astype(np.float16)  # integer-valued, exact
            im[f"wt{c}"] = np.ascontiguousarray(sl)
            g += pairs
        in_maps.append(im)
    return in_maps


def gather_output(results, W_my, b_my, W_opp, b_opp):
    """results: per-core {'outA','outB': [1,256]} int-unit partials."""
    s_my, s_opp = _quant_scales(W_my, W_opp)
    acc = np.zeros(512, np.float32)
    for core in range(N_CORES):
        acc[:256] += np.asarray(results[core]["outA"], np.float32)[0]
        acc[256:] += np.asarray(results[core]["outB"], np.float32)[0]
    acc[:256] *= s_my
    acc[256:] *= s_opp
    bcat = np.concatenate(
        [np.asarray(b_my, np.float32), np.asarray(b_opp, np.float32)]
    )
    return np.maximum(acc + bcat, 0.0)


def run_on_hw(in_maps, trace=False, **kwargs):
    from concourse.bass_utils import run_bass_kernel_spmd

    nc = _get_nc()
    return run_bass_kernel_spmd(
        nc, in_maps, core_ids=list(range(N_CORES)), trace=trace, **kwargs
    )


def kernel(input, W_my, b_my, W_opp, b_opp):
    in_maps = make_in_maps(input, W_my, b_my, W_opp, b_opp)
    res = run_on_hw(in_maps)
    return gather_output(res.results, W_my, b_my, W_opp, b_opp)


# revision 19
# speedup vs baseline: 1.0521x; 1.0249x over previous
"""HalfKP input layer (dual GEMV + bias + relu) on 8 Trainium2 NeuronCores.

out[512] = concat(relu(W_my @ x[:41024] + b_my), relu(W_opp @ x[41024:] + b_opp))

Memory-roofline kernel: the 84 MB f32 weight stream is the whole cost.
Weights ship as int8 with one global scale per side (5.5e-3 end-to-end rel
err) and each core's 2.7 MB shard is expanded int8 -> fp16 in flight by
the SWDGE cast-DMA: HBM reads halve (killing cross-core HBM-stack
contention) and the stream is bound only by the core-private SBUF AXI
write side (~435 GB/s, ~12.4 us).

Sharding: k-parallel.  Every core holds all 512 output rows and 1/8 of
the contraction (5128 k per side, zero-padded to 41 blocks of 128).
Compute rides the PE in GEMV mode with fat moving tiles: for k-block g
and side s, matmul(stationary = x_block [128,1], moving = W_block^T
[128,256]) accumulates into a per-side PSUM bank.  Dep-free warm-up
matmuls flip the PE HAM clock-gate to 2.4 GHz during DMA spin-up so real
matmuls run at ~110 ns, fully hidden under the stream.  Cores return raw
f32 integer-unit partials; the host applies the int8 scales, sums the 8
partials, adds bias, applies relu.
"""

import numpy as np

K = 41024              # features per side
KSL = K // 8           # 5128 contraction elems per core per side
NBS = 41               # 128-elem k-blocks per side per core (5248, padded)
BK = NBS * 128
N_CORES = 8
# W chunks in block-pairs (one pair = my+opp blocks = 1 KB/partition fp16
# in SBUF).  The first chunks ride the sync HWDGE ring as integer-valued
# fp16 (data flowing ~2 us before the SWDGE Q7 finishes descriptor-gen);
# the rest stream as int8 through the SWDGE cast-DMA.  Small tail chunk
# keeps the last-semaphore exposure off the critical path.
SYNC_PAIRS = []              # fp16-direct chunks (HWDGE); a hybrid ring
                             # split measured no better than pure SWDGE
SWDGE_PAIRS = [4, 8, 8, 8, 8, 4, 1]  # int8->fp16 cast chunks (SWDGE)
                                     # (>8-pair SWDGE chunks wedge the device)
CHUNK_PAIRS = SYNC_PAIRS + SWDGE_PAIRS
assert sum(CHUNK_PAIRS) == NBS
N_WARMUP = 9  # dummy N=512 matmuls that flip the PE HAM clock-gate to
              # 2.4 GHz during DMA spin-up, so real matmuls run warm

_compiled = None


def _build_nc():
    import concourse.bacc as bacc
    import concourse.mybir as mybir
    import concourse.tile as tile

    F32 = mybir.dt.float32
    F16 = mybir.dt.float16
    I8 = mybir.dt.int8

    nc = bacc.Bacc("TRN2", target_bir_lowering=False, debug=False)

    wt_d = [
        nc.dram_tensor(
            f"wt{c}",
            [128, pairs * 512],
            F16 if c < len(SYNC_PAIRS) else I8,
            kind="ExternalInput",
        )
        for c, pairs in enumerate(CHUNK_PAIRS)
    ]
    xq_d = nc.dram_tensor("xq", [128, 2 * NBS], F16, kind="ExternalInput")
    outA_d = nc.dram_tensor("outA", [1, 256], F32, kind="ExternalOutput")
    outB_d = nc.dram_tensor("outB", [1, 256], F32, kind="ExternalOutput")

    with tile.TileContext(nc) as tc:
        with (
            tc.tile_pool(name="const", bufs=1) as constp,
            tc.tile_pool(name="w", bufs=len(CHUNK_PAIRS) + 1) as wp,
            tc.tile_pool(name="ps", bufs=1, space="PSUM") as psp,
        ):
            # x rides the scalar (ACT) HWDGE ring; the W stream runs on the
            # SWDGE (gpsimd) ring which also does the int8->fp16 expansion
            xq = constp.tile([128, 2 * NBS], F16, tag="xq")
            nc.scalar.dma_start(xq[:], xq_d[:])

            # full-bank tiles so the two accumulation groups can't share a
            # bank (a group's start=True clears its whole bank)
            psA = psp.tile([1, 512], F32, tag="psA")
            psB = psp.tile([1, 512], F32, tag="psB")

            # PE warm-up: dep-free matmuls on zeroed scratch keep the PE
            # busy from t=0 so HAM un-throttles before real work arrives
            warm_w = constp.tile([128, 512], F16, tag="warm_w")
            warm_x = constp.tile([128, 1], F16, tag="warm_x")
            warm_ps = psp.tile([1, 512], F32, tag="warm_ps")
            nc.vector.memset(warm_w[:], 0)
            nc.vector.memset(warm_x[:], 0)
            for _ in range(N_WARMUP):
                nc.tensor.matmul(
                    warm_ps[:], lhsT=warm_x[:], rhs=warm_w[:],
                    start=True, stop=True,
                )

            g = 0
            for c, pairs in enumerate(CHUNK_PAIRS):
                w_sb = wp.tile([128, pairs * 512], F16, tag="w")
                if c < len(SYNC_PAIRS):
                    nc.sync.dma_start(w_sb[:], wt_d[c][:])
                else:
                    nc.gpsimd.dma_start(w_sb[:], wt_d[c][:])  # int8 -> fp16
                for j in range(pairs):
                    for s, ps in ((0, psA), (1, psB)):
                        nc.tensor.matmul(
                            ps[:, 0:256],
                            lhsT=xq[:, 2 * (g + j) + s : 2 * (g + j) + s + 1],
                            rhs=w_sb[:, j * 512 + s * 256 : j * 512 + (s + 1) * 256],
                            start=(g + j == 0),
                            stop=(g + j == NBS - 1),
                        )
                g += pairs

            # different PSUM banks -> ScalarE and VectorE copy in parallel,
            # then each half goes out on its own HWDGE ring so the two
            # completion receipts overlap
            outA_sb = constp.tile([1, 256], F32, tag="outA")
            outB_sb = constp.tile([1, 256], F32, tag="outB")
            nc.scalar.copy(outA_sb[:], psA[:, 0:256])
            nc.vector.tensor_scalar_add(outB_sb[:], psB[:, 0:256], 0.0)
            nc.scalar.dma_start(outA_d[:], outA_sb[:])
            nc.sync.dma_start(outB_d[:], outB_sb[:])

    nc.compile()
    return nc


def _get_nc():
    global _compiled
    if _compiled is None:
        _compiled = _build_nc()
    return _compiled


def _quant_scales(W_my, W_opp):
    return (
        np.abs(np.asarray(W_my, np.float32)).max() / 127.0,
        np.abs(np.asarray(W_opp, np.float32)).max() / 127.0,
    )


def make_in_maps(input, W_my, b_my, W_opp, b_opp):
    """Host-side sharding: per-core input dicts."""
    x = np.asarray(input, np.float32)
    s_my, s_opp = _quant_scales(W_my, W_opp)
    Wq = [
        np.round(np.asarray(W_my, np.float32) / s_my).astype(np.int8),
        np.round(np.asarray(W_opp, np.float32) / s_opp).astype(np.int8),
    ]
    xs = [x[:K], x[K:]]

    in_maps = []
    for core in range(N_CORES):
        ksl = slice(core * KSL, (core + 1) * KSL)
        # wt[p, (2g+s)*256 + j] = Wq_s[j, core*KSL + g*128 + p]
        wt = np.zeros((128, NBS, 2, 256), np.int8)
        xq = np.zeros((128, NBS, 2), np.float16)
        for s in (0, 1):
            Wp = np.zeros((BK, 256), np.int8)
            Wp[:KSL] = Wq[s][:, ksl].T  # [KSL, 256]
            wt[:, :, s, :] = Wp.reshape(NBS, 128, 256).transpose(1, 0, 2)
            xp = np.zeros(BK, np.float16)
            xp[:KSL] = xs[s][ksl]
            xq[:, :, s] = xp.reshape(NBS, 128).T
        wt = wt.reshape(128, NBS * 512)
        im = {"xq": np.ascontiguousarray(xq.reshape(128, 2 * NBS))}
        g = 0
        for c, pairs in enumerate(CHUNK_PAIRS):
            sl = wt[:, g * 512 : (g + pairs) * 512]
            if c < len(SYNC_PAIRS):
                sl = sl.astype(np.float16)  # integer-valued, exact
            im[f"wt{c}"] = np.ascontiguousarray(sl)
            g += pairs
        in_maps.append(im)
    return in_maps


def gather_output(results, W_my, b_my, W_opp, b_opp):
    """results: per-core {'outA','outB': [1,256]} int-unit partials."""
    s_my, s_opp = _quant_scales(W_my, W_opp)
    acc = np.zeros(512, np.float32)
    for core in range(N_CORES):
        acc[:256] += np.asarray(results[core]["outA"], np.float32)[0]
        acc[256:] += np.asarray(results[core]["outB"], np.float32)[0]
    acc[:256] *= s_my
    acc[256:] *= s_opp
    bcat = np.concatenate(
        [np.asarray(b_my, np.float32), np.asarray(b_opp, np.float32)]
    )
    return np.maximum(acc + bcat, 0.0)


def run_on_hw(in_maps, trace=False, **kwargs):
    from concourse.bass_utils import run_bass_kernel_spmd

    nc = _get_nc()
    return run_bass_kernel_spmd(
        nc, in_maps, core_ids=list(range(N_CORES)), trace=trace, **kwargs
    )


def kernel(input, W_my, b_my, W_opp, b_opp):
    in_maps = make_in_maps(input, W_my, b_my, W_opp, b_opp)
    res = run_on_hw(in_maps)
    return gather_output(res.results, W_my, b_my, W_opp, b_opp)


# revision 20
# speedup vs baseline: 1.0558x; 1.0035x over previous
"""HalfKP input layer (dual GEMV + bias + relu) on 8 Trainium2 NeuronCores.

out[512] = concat(relu(W_my @ x[:41024] + b_my), relu(W_opp @ x[41024:] + b_opp))

Memory-roofline kernel: the 84 MB f32 weight stream is the whole cost.
Weights ship as int8 with one global scale per side (5.5e-3 end-to-end rel
err) and each core's 2.7 MB shard is expanded int8 -> fp16 in flight by
the SWDGE cast-DMA: HBM reads halve (killing cross-core HBM-stack
contention) and the stream is bound only by the core-private SBUF AXI
write side (~435 GB/s, ~12.4 us).

Sharding: k-parallel.  Every core holds all 512 output rows and 1/8 of
the contraction (5128 k per side, zero-padded to 41 blocks of 128).
Compute rides the PE in GEMV mode with fat moving tiles: for k-block g
and side s, matmul(stationary = x_block [128,1], moving = W_block^T
[128,256]) accumulates into a per-side PSUM bank.  Dep-free warm-up
matmuls flip the PE HAM clock-gate to 2.4 GHz during DMA spin-up so real
matmuls run at ~110 ns, fully hidden under the stream.  Cores return raw
f32 integer-unit partials; the host applies the int8 scales, sums the 8
partials, adds bias, applies relu.
"""

import numpy as np

K = 41024              # features per side
KSL = K // 8           # 5128 contraction elems per core per side
NBS = 41               # 128-elem k-blocks per side per core (5248, padded)
BK = NBS * 128
N_CORES = 8
# W chunks in block-pairs (one pair = my+opp blocks = 1 KB/partition fp16
# in SBUF).  The first chunks ride the sync HWDGE ring as integer-valued
# fp16 (data flowing ~2 us before the SWDGE Q7 finishes descriptor-gen);
# the rest stream as int8 through the SWDGE cast-DMA.  Small tail chunk
# keeps the last-semaphore exposure off the critical path.
SYNC_PAIRS = []              # fp16-direct chunks (HWDGE); a hybrid ring
                             # split measured no better than pure SWDGE
SWDGE_PAIRS = [4, 8, 8, 8, 8, 4, 1]  # int8->fp16 cast chunks (SWDGE)
                                     # (>8-pair SWDGE chunks wedge the device)
CHUNK_PAIRS = SYNC_PAIRS + SWDGE_PAIRS
assert sum(CHUNK_PAIRS) == NBS
N_WARMUP = 8  # dummy N=512 matmuls that flip the PE HAM clock-gate to
              # 2.4 GHz during DMA spin-up, so real matmuls run warm

_compiled = None


def _build_nc():
    import concourse.bacc as bacc
    import concourse.mybir as mybir
    import concourse.tile as tile

    F32 = mybir.dt.float32
    F16 = mybir.dt.float16
    I8 = mybir.dt.int8

    nc = bacc.Bacc("TRN2", target_bir_lowering=False, debug=False)

    wt_d = [
        nc.dram_tensor(
            f"wt{c}",
            [128, pairs * 512],
            F16 if c < len(SYNC_PAIRS) else I8,
            kind="ExternalInput",
        )
        for c, pairs in enumerate(CHUNK_PAIRS)
    ]
    xq_d = nc.dram_tensor("xq", [128, 2 * NBS], F16, kind="ExternalInput")
    outA_d = nc.dram_tensor("outA", [1, 256], F32, kind="ExternalOutput")
    outB_d = nc.dram_tensor("outB", [1, 256], F32, kind="ExternalOutput")

    with tile.TileContext(nc) as tc:
        with (
            tc.tile_pool(name="const", bufs=1) as constp,
            tc.tile_pool(name="w", bufs=len(CHUNK_PAIRS) + 1) as wp,
            tc.tile_pool(name="ps", bufs=1, space="PSUM") as psp,
        ):
            # x rides the scalar (ACT) HWDGE ring; the W stream runs on the
            # SWDGE (gpsimd) ring which also does the int8->fp16 expansion
            xq = constp.tile([128, 2 * NBS], F16, tag="xq")
            nc.scalar.dma_start(xq[:], xq_d[:])

            # full-bank tiles so the two accumulation groups can't share a
            # bank (a group's start=True clears its whole bank)
            psA = psp.tile([1, 512], F32, tag="psA")
            psB = psp.tile([1, 512], F32, tag="psB")

            # PE warm-up: dep-free matmuls on zeroed scratch keep the PE
            # busy from t=0 so HAM un-throttles before real work arrives
            warm_w = constp.tile([128, 512], F16, tag="warm_w")
            warm_x = constp.tile([128, 1], F16, tag="warm_x")
            warm_ps = psp.tile([1, 512], F32, tag="warm_ps")
            nc.vector.memset(warm_w[:], 0)
            nc.vector.memset(warm_x[:], 0)
            for _ in range(N_WARMUP):
                nc.tensor.matmul(
                    warm_ps[:], lhsT=warm_x[:], rhs=warm_w[:],
                    start=True, stop=True,
                )

            g = 0
            for c, pairs in enumerate(CHUNK_PAIRS):
                w_sb = wp.tile([128, pairs * 512], F16, tag="w")
                if c < len(SYNC_PAIRS):
                    nc.sync.dma_start(w_sb[:], wt_d[c][:])
                else:
                    nc.gpsimd.dma_start(w_sb[:], wt_d[c][:])  # int8 -> fp16
                for j in range(pairs):
                    for s, ps in ((0, psA), (1, psB)):
                        nc.tensor.matmul(
                            ps[:, 0:256],
                            lhsT=xq[:, 2 * (g + j) + s : 2 * (g + j) + s + 1],
                            rhs=w_sb[:, j * 512 + s * 256 : j * 512 + (s + 1) * 256],
                            start=(g + j == 0),
                            stop=(g + j == NBS - 1),
                        )
                g += pairs

            # different PSUM banks -> ScalarE and VectorE copy in parallel,
            # then each half goes out on its own HWDGE ring so the two
            # completion receipts overlap
            outA_sb = constp.tile([1, 256], F32, tag="outA")
            outB_sb = constp.tile([1, 256], F32, tag="outB")
            nc.scalar.copy(outA_sb[:], psA[:, 0:256])
            nc.vector.tensor_scalar_add(outB_sb[:], psB[:, 0:256], 0.0)
            nc.scalar.dma_start(outA_d[:], outA_sb[:])
            nc.sync.dma_start(outB_d[:], outB_sb[:])

    nc.compile()
    return nc


def _get_nc():
    global _compiled
    if _compiled is None:
        _compiled = _build_nc()
    return _compiled


def _quant_scales(W_my, W_opp):
    return (
        np.abs(np.asarray(W_my, np.float32)).max() / 127.0,
        np.abs(np.asarray(W_opp, np.float32)).max() / 127.0,
    )


def make_in_maps(input, W_my, b_my, W_opp, b_opp):
    """Host-side sharding: per-core input dicts."""
    x = np.asarray(input, np.float32)
    s_my, s_opp = _quant_scales(W_my, W_opp)
    Wq = [
        np.round(np.asarray(W_my, np.float32) / s_my).astype(np.int8),
        np.round(np.asarray(W_opp, np.float32) / s_opp).astype(np.int8),
    ]
    xs = [x[:K], x[K:]]

    in_maps = []
    for core in range(N_CORES):
        ksl = slice(core * KSL, (core + 1) * KSL)
        # wt[p, (2g+s)*256 + j] = Wq_s[j, core*KSL + g*128 + p]
        wt = np.zeros((128, NBS, 2, 256), np.int8)
        xq = np.zeros((128, NBS, 2), np.float16)
        for s in (0, 1):
            Wp = np.zeros((BK, 256), np.int8)
            Wp[:KSL] = Wq[s][:, ksl].T  # [KSL, 256]
            wt[:, :, s, :] = Wp.reshape(NBS, 128, 256).transpose(1, 0, 2)
            xp = np.zeros(BK, np.float16)
            xp[:KSL] = xs[s][ksl]
            xq[:, :, s] = xp.reshape(NBS, 128).T
        wt = wt.reshape(128, NBS * 512)
        im = {"xq": np.ascontiguousarray(xq.reshape(128, 2 * NBS))}
        g = 0
        for c, pairs in enumerate(CHUNK_PAIRS):
            sl = wt[:, g * 512 : (g + pairs) * 512]
            if c < len(SYNC_PAIRS):
                sl = sl.astype(np.float16)  # integer-valued, exact
            im[f"wt{c}"] = np.ascontiguousarray(sl)
            g += pairs
        in_maps.append(im)
    return in_maps


def gather_output(results, W_my, b_my, W_opp, b_opp):
    """results: per-core {'outA','outB': [1,256]} int-unit partials."""
    s_my, s_opp = _quant_scales(W_my, W_opp)
    acc = np.zeros(512, np.float32)
    for core in range(N_CORES):
        acc[:256] += np.asarray(results[core]["outA"], np.float32)[0]
        acc[256:] += np.asarray(results[core]["outB"], np.float32)[0]
    acc[:256] *= s_my
    acc[256:] *= s_opp
    bcat = np.concatenate(
        [np.asarray(b_my, np.float32), np.asarray(b_opp, np.float32)]
    )
    return np.maximum(acc + bcat, 0.0)


def run_on_hw(in_maps, trace=False, **kwargs):
    from concourse.bass_utils import run_bass_kernel_spmd

    nc = _get_nc()
    return run_bass_kernel_spmd(
        nc, in_maps, core_ids=list(range(N_CORES)), trace=trace, **kwargs
    )


def kernel(input, W_my, b_my, W_opp, b_opp):
    in_maps = make_in_maps(input, W_my, b_my, W_opp, b_opp)
    res = run_on_hw(in_maps)
    return gather_output(res.results, W_my, b_my, W_opp, b_opp)
